# revision 36
# baseline (speedup 1.0000x reference)
"""Multi-head self-attention Trainium2 kernel (8-core SPMD, head-parallel).

Problem: B=2, N=4096, D=768, H=12 heads, head_dim=64, fp32 in/out.

Sharding (Megatron-style tensor parallel over (batch, head) pairs):
  - 24 (b, h) pairs across 8 cores -> 3 heads per core, one batch per core.
  - Each core: QKV projection for its 3 heads, full attention, and a
    row-parallel slice of the output projection producing a partial
    [768, 4096] transposed output.  Host sums the 4 partials per batch,
    adds b_proj, transposes back.

Perf design (v3):
  - All matmul operands bf16 (the PE moving-operand port is byte-limited;
    fp32 streams at half rate).  PSUM accumulation stays fp32.
  - Every QK matmul is issued as a ROW-TILED PAIR on partitions 0:64 /
    64:128 so two K=64 contractions run concurrently (full-array
    activity keeps the HAM clock gate at 2.4 GHz; at half clock the
    pairs serialize and the whole kernel runs 2x slower -- a bistable
    trap).  Heads 0/1 pair with each other; head 2 pairs consecutive
    key chunks using partition-shifted copies of its Q^T/K^T.
  - The scalar engine's one-time exp ACT_TABLE_LOAD is pre-triggered in
    phase 1 so the phase transition has no PE bubble (a >1us bubble at
    the wrong moment re-throttles the clock gate and it never recovers).
  - Softmax exp is split across both engines per pair: ScalarE table exp
    for one tile, VectorE exp2 bit trick for the other (i16 = S*A + B;
    bitcast i16 -> bf16 == 2^(S*SCALE*log2e) to +-3%, averages out in
    softmax).
  - V tiles carry 97 columns (64 dims + ones-column for the row-sum
    trick + pad to 4 PE column groups).
  - Normalization: per-query row sums are collected into one [12, QB]
    tile by tiny SBUF->SBUF DMAs; ONE exact reciprocal (the 1-lane DVE
    reciprocal costs 6.5us, so never run 12 of them mid-attention);
    1/sum rows are broadcast to 64 partitions by a one-hot-select bf16
    matmul in phase 4 where PSUM banks are free.
  - Output projection fuses heads 0+1 into K=128 matmuls (head 1's rows
    partition-shifted by a small SBUF->SBUF DMA).
"""

import numpy as np
import ml_dtypes

import concourse.bass as bass
import concourse.bacc as bacc
import concourse.mybir as mybir
import concourse.tile as tile
from concourse.bass_utils import run_bass_kernel_spmd

F32 = mybir.dt.float32
BF16 = mybir.dt.bfloat16
I16 = mybir.dt.int16

B, N, D = 2, 4096, 768
H, HD = 12, 64
SCALE = HD ** -0.5
NCORES = 8
NH = 3            # heads per core
DC = D // 128     # 6 contraction chunks for the qkv projection
NB = N // 512     # 8 column blocks of 512
KC = N // 128     # 32 key chunks
QB = 1024         # query block (softmax/AV granularity)
NQB = N // QB     # 4 query blocks
VW = 97           # V tile width: 64 dims + ones col + pad (4 col grps)
NT = NH * NQB     # 12 normalization tails

# exp2 bit trick: bitcast_bf16(int16(s*A + B)) ~= exp(s*SCALE)
# (DVE fp32->int16 output conversion is exact round-to-nearest on HW)
EXP2_A = SCALE * np.log2(np.e) * 128.0
EXP2_B = (127.0 - 0.043) * 128.0

# packed weight column layout (see _pack_core_inputs):
#   [qa(128) | ka(128) | qb(64) | kb(64) | va(128) | vb(64)]  -> 576 cols
_OFF_QA, _OFF_KA, _OFF_QB, _OFF_KB, _OFF_VA, _OFF_VB = 0, 128, 256, 320, 384, 512

AF = mybir.ActivationFunctionType


def build_module(debug_dump: bool = False) -> bass.Bass:
    nc = bacc.Bacc("TRN2", target_bir_lowering=False, debug=False)

    dbg = {}
    if debug_dump:
        dbg["aT01"] = nc.declare_dram_parameter("d_aT01", [128, N], BF16, isOutput=True)
        dbg["aT2"] = nc.declare_dram_parameter("d_aT2", [HD, N], BF16, isOutput=True)

    xT = nc.declare_dram_parameter("xT", [D, N], BF16, isOutput=False)
    wp = nc.declare_dram_parameter("wp", [DC, 128, 576], BF16, isOutput=False)
    bp = nc.declare_dram_parameter("bp", [128, 6], F32, isOutput=False)
    wproj01 = nc.declare_dram_parameter("wproj01", [128, D], BF16, isOutput=False)
    wproj2 = nc.declare_dram_parameter("wproj2", [HD, D], BF16, isOutput=False)
    ident = nc.declare_dram_parameter("ident", [128, 128], BF16, isOutput=False)
    sel = nc.declare_dram_parameter("sel", [NT, NT, HD], BF16, isOutput=False)
    outT = nc.declare_dram_parameter("outT", [D, N], F32, isOutput=True)

    with tile.TileContext(nc) as tc:
        with (
            tc.tile_pool(name="consts", bufs=1) as consts,
            tc.tile_pool(name="qkstore", bufs=1) as qkstore,
            tc.tile_pool(name="vstore", bufs=1) as vstore,
            tc.tile_pool(name="sums", bufs=1) as sums,
        ):
            # ---- persistent SBUF tensors -------------------------------
            w_sb = consts.tile([128, DC, 576], BF16)
            nc.sync.dma_start(w_sb[:], wp.rearrange("c p m -> p c m"))
            b_sb = consts.tile([128, 6], F32)
            nc.sync.dma_start(b_sb[:], bp[:])
            wproj01_sb = consts.tile([128, D], BF16)
            nc.sync.dma_start(wproj01_sb[:], wproj01[:])
            wproj2_sb = consts.tile([HD, D], BF16)
            nc.sync.dma_start(wproj2_sb[:], wproj2[:])
            ident_sb = consts.tile([128, 128], BF16)
            nc.sync.dma_start(ident_sb[:], ident[:])
            sel_sb = consts.tile([NT, NT, HD], BF16)
            nc.sync.dma_start(sel_sb[:], sel[:])
            # pre-trigger the one-time exp ACT_TABLE_LOAD (~2.7us) NOW so
            # it does not bubble the PE at the phase-1 -> attention
            # transition (a bubble there re-throttles the clock gate).
            scratch = consts.tile([1, 6], BF16)
            nc.scalar.activation(scratch[:], b_sb[0:1, :], AF.Exp, scale=0.001)

            # Q^T/K^T: heads 0,1 packed on partitions [0:64]/[64:128] of
            # the "a" tiles; head 2 duplicated on both halves of the "b"
            # tiles (so its QK matmuls can also issue as row-tiled pairs).
            qTa = qkstore.tile([128, N], BF16)
            kTa = qkstore.tile([128, N], BF16)
            qTb = qkstore.tile([128, N], BF16)
            kTb = qkstore.tile([128, N], BF16)
            # V in [key, dim] layout; col 64 = 1.0 (row-sum trick)
            v_sb = [vstore.tile([128, KC, VW], BF16, name=f"v_sb{h}")
                    for h in range(NH)]
            for h in range(NH):
                nc.vector.memset(v_sb[h][:], 0.0)
                nc.vector.memset(v_sb[h][:, :, HD:HD + 1], 1.0)
            # softmax denominators, one row per (head, qb) tail; ONE
            # batched reciprocal at phase-4 start (never 12 separate 6.5us
            # single-lane reciprocals stalling the vector engine mid-flight)
            sums_all = sums.tile([NT, QB], F32)

            # ---- phase 1: QKV projection + interleaved V transposes -----
            with (
                tc.tile_pool(name="xpool", bufs=3) as xpool,
                tc.tile_pool(name="vtpool", bufs=1) as vtpool,
                tc.tile_pool(name="prjpsum", bufs=4, space="PSUM") as prjpsum,
            ):
                vTa = vtpool.tile([128, N], BF16)   # V^T heads 0,1
                vTb = vtpool.tile([HD, N], BF16)    # V^T head 2
                groups = [
                    (qTa, _OFF_QA, 128, 0),
                    (kTa, _OFF_KA, 128, 1),
                    (qTb, _OFF_QB, HD, 2),
                    (kTb, _OFF_KB, HD, 3),
                    (vTa, _OFF_VA, 128, 4),
                    (vTb, _OFF_VB, HD, 5),
                ]
                with tc.tile_pool(name="tppsum", bufs=4, space="PSUM") as tppsum:
                    for nb in range(NB):
                        xt = xpool.tile([128, DC, 512], BF16)
                        nc.sync.dma_start(
                            xt[:],
                            xT.rearrange("(c p) n -> p c n", p=128)[
                                :, :, nb * 512:(nb + 1) * 512
                            ],
                        )
                        for dest, off, m, bcol in groups:
                            pp = prjpsum.tile([128, 512], F32, tag="pp")
                            for c in range(DC):
                                nc.tensor.matmul(
                                    pp[0:m, :],
                                    w_sb[:, c, off:off + m],
                                    xt[:, c, :],
                                    start=(c == 0),
                                    stop=(c == DC - 1),
                                )
                            # bias-add on ScalarE (idle in phase 1; the
                            # vector engine is busy during attention)
                            nc.scalar.activation(
                                dest[0:m, nb * 512:(nb + 1) * 512],
                                pp[0:m, :],
                                AF.Identity,
                                bias=b_sb[0:m, bcol:bcol + 1],
                            )
                        # head 2's Q^T/K^T duplicated to partitions 64:128
                        # (row-tile pairing needs both array halves fed)
                        nc.sync.dma_start(
                            qTb[HD:128, nb * 512:(nb + 1) * 512],
                            qTb[0:HD, nb * 512:(nb + 1) * 512])
                        nc.sync.dma_start(
                            kTb[HD:128, nb * 512:(nb + 1) * 512],
                            kTb[0:HD, nb * 512:(nb + 1) * 512])
                        # transpose this block's V^T columns into V tiles
                        for h in range(NH):
                            if h < 2:
                                src, base = vTa, 64 * h
                            else:
                                src, base = vTb, 0
                            for k in range(4 * nb, 4 * nb + 4):
                                tp = tppsum.tile([128, HD], BF16, tag="tp")
                                nc.tensor.transpose(
                                    tp[:],
                                    src[base:base + HD, k * 128:(k + 1) * 128],
                                    ident_sb[base:base + HD, base:base + HD],
                                )
                                nc.vector.tensor_copy(v_sb[h][:, k, 0:HD], tp[:])

            # ---- phase 3: attention ------------------------------------
            with tc.tile_pool(name="attnstore", bufs=1) as attnstore:
                # aT01: head0 rows on partitions 0:64, head1 on 64:128
                # (head1 arrives via partition-shift DMA from its tail)
                aT01 = attnstore.tile([128, N], BF16)
                aT2 = attnstore.tile([HD, N], BF16)
                ctx_up = tc.tile_pool(name="upool", bufs=1)
                upool = ctx_up.__enter__()  # must outlive phase 4 reads
                ctx_ra = tc.tile_pool(name="rall", bufs=1)
                rall = ctx_ra.__enter__()
                r_all = rall.tile([NT, QB], F32)
                r_bf = rall.tile([NT, QB], BF16)
                tails: dict = {}
                with (
                    tc.tile_pool(name="ppool", bufs=4) as ppool,
                    tc.tile_pool(name="stage", bufs=4) as stage,
                    tc.tile_pool(name="spsum", bufs=2, space="PSUM") as spsum,
                    tc.tile_pool(name="avpsum", bufs=2, space="PSUM") as avpsum,
                ):
                    def qk_mm(s, kT, qT, lo, k, qb):
                        """One QK matmul on array rows [lo, lo+64)."""
                        q0 = qb * QB
                        for x2 in range(QB // 512):
                            nc.tensor.matmul(
                                s[:, x2 * 512:(x2 + 1) * 512],
                                kT[lo:lo + HD, k * 128:(k + 1) * 128],
                                qT[lo:lo + HD, q0 + x2 * 512:q0 + (x2 + 1) * 512],
                                start=True, stop=True,
                            )

                    def exp_act(s):
                        p = ppool.tile([128, QB], BF16, tag="p", name="pa")
                        nc.scalar.activation(p[:], s[:], AF.Exp, scale=SCALE)
                        return p

                    def exp_dve(s):
                        p = ppool.tile([128, QB], BF16, tag="p", name="pv")
                        nc.vector.tensor_scalar(
                            p.bitcast(I16)[:], s[:],
                            EXP2_A, EXP2_B,
                            mybir.AluOpType.mult, mybir.AluOpType.add,
                        )
                        return p

                    def av_mm(av, h, k, p):
                        for x2 in range(QB // 512):
                            nc.tensor.matmul(
                                av[:, x2 * 512:(x2 + 1) * 512],
                                v_sb[h][:, k, :],
                                p[:, x2 * 512:(x2 + 1) * 512],
                                start=(k == 0),
                                stop=(k == KC - 1),
                            )

                    def begin_tail(h, qb, av):
                        # rows 0:64 + the rowsum row off PSUM (bf16 is
                        # plenty for the numerator); the fp32 sum row is
                        # staged and DMA'd into sums_all for a batched
                        # reciprocal.  The two copies go to DIFFERENT
                        # engines so a section boundary only delays each
                        # engine's next exp by one op.
                        u65 = upool.tile([HD + 1, QB], BF16, tag="u",
                                         bufs=NT, name="u65")
                        nc.vector.tensor_copy(u65[:], av[0:HD + 1, :])
                        srow = stage.tile([1, QB], F32, tag="srow", name="srow")
                        nc.scalar.copy(srow[:], av[HD:HD + 1, :])
                        idx = qb * NH + h
                        nc.sync.dma_start(sums_all[idx:idx + 1, :], srow[:])
                        tails[(h, qb)] = u65

                    def warm_burst(n_mm):
                        # dense full-array matmuls whose results are
                        # discarded: forces the HAM activity monitor to
                        # un-throttle the PE clock (K=8/8) before a phase
                        # whose own matmul mix is too sparse to trigger it.
                        scr = spsum.tile([128, QB], F32, tag="s", name="scr")
                        for i in range(n_mm):
                            nc.tensor.matmul(
                                scr[:, 0:512], w_sb[:, 0, 0:128],
                                qTa[:, 0:512], start=True, stop=True,
                            )

                    warm_burst(16)
                    for qb in range(NQB):
                        # ---- heads 0,1: row-tiled pairs across heads
                        av0 = avpsum.tile([VW, QB], F32, tag="av", name="av0")
                        av1 = avpsum.tile([VW, QB], F32, tag="av", name="av1")
                        pend = None
                        for k in range(KC):
                            s0 = spsum.tile([128, QB], F32, tag="s", name="s0")
                            s1 = spsum.tile([128, QB], F32, tag="s", name="s1")
                            qk_mm(s0, kTa, qTa, 0, k, qb)
                            qk_mm(s1, kTa, qTa, HD, k, qb)
                            p0 = exp_act(s0)
                            p1 = exp_dve(s1)
                            if pend is not None:
                                av_mm(av0, 0, k - 1, pend[0])
                                av_mm(av1, 1, k - 1, pend[1])
                            pend = (p0, p1)
                        av_mm(av0, 0, KC - 1, pend[0])
                        av_mm(av1, 1, KC - 1, pend[1])
                        begin_tail(0, qb, av0)
                        begin_tail(1, qb, av1)

                        # ---- head 2: row-tiled pairs across chunk parity
                        av2 = avpsum.tile([VW, QB], F32, tag="av", name="av2")
                        pend2 = None
                        for kk in range(0, KC, 2):
                            s0 = spsum.tile([128, QB], F32, tag="s", name="s2e")
                            s1 = spsum.tile([128, QB], F32, tag="s", name="s2o")
                            qk_mm(s0, kTb, qTb, 0, kk, qb)
                            qk_mm(s1, kTb, qTb, HD, kk + 1, qb)
                            p0 = exp_act(s0)
                            p1 = exp_dve(s1)
                            if pend2 is not None:
                                av_mm(av2, 2, kk - 2, pend2[0])
                                av_mm(av2, 2, kk - 1, pend2[1])
                            pend2 = (p0, p1)
                        av_mm(av2, 2, KC - 2, pend2[0])
                        av_mm(av2, 2, KC - 1, pend2[1])
                        begin_tail(2, qb, av2)

                # ---- phase 4: remaining reciprocal, tails, projection --
                # interleaved per query block so the PE stream stays dense
                # (full-util projection matmuls between the tiny broadcast
                # matmuls) and never re-throttles.
                with (
                    tc.tile_pool(name="opool", bufs=3) as opool,
                    tc.tile_pool(name="a1pool", bufs=2) as a1pool,
                    tc.tile_pool(name="prpsum", bufs=4, space="PSUM") as prpsum,
                    tc.tile_pool(name="bpspsum", bufs=2, space="PSUM") as bpspsum,
                ):
                    nc.vector.reciprocal(r_all[:], sums_all[:])
                    nc.vector.tensor_copy(r_bf[:], r_all[:])

                    def warm_burst4(n_mm):
                        scr = prpsum.tile([128, 512], F32, tag="pr",
                                          name="scr")
                        for i in range(n_mm):
                            nc.tensor.matmul(
                                scr[:], w_sb[:, 0, 0:128], qTa[:, 0:512],
                                start=True, stop=True,
                            )

                    # long enough to keep the PE busy (and the clock gate
                    # warm) for the full reciprocal + bf16-copy latency
                    warm_burst4(36)

                    def do_tail(h, qb):
                        q0 = qb * QB
                        u65 = tails[(h, qb)]
                        idx = qb * NH + h
                        bps = bpspsum.tile([HD, QB], F32, tag="bps",
                                           name="bps")
                        for x2 in range(QB // 512):
                            nc.tensor.matmul(
                                bps[:, x2 * 512:(x2 + 1) * 512],
                                sel_sb[:, idx, :],
                                r_bf[:, x2 * 512:(x2 + 1) * 512],
                                start=True, stop=True,
                            )
                        if h == 0:
                            dst = aT01[0:HD, q0:q0 + QB]
                        elif h == 2:
                            dst = aT2[:, q0:q0 + QB]
                        else:
                            dst = a1pool.tile([HD, QB], BF16, tag="a1",
                                              name="a1")[:]
                        nc.vector.tensor_mul(dst, u65[0:HD, :], bps[:])
                        if h == 1:
                            # partition-shift head1 rows to 64:128 so the
                            # projection fuses heads 0+1 at K=128
                            nc.sync.dma_start(aT01[HD:128, q0:q0 + QB], dst)

                    def do_proj(nb):
                        for oc in range(DC):
                            pr = prpsum.tile([128, 512], F32, tag="pr")
                            nc.tensor.matmul(
                                pr[:],
                                wproj01_sb[:, oc * 128:(oc + 1) * 128],
                                aT01[:, nb * 512:(nb + 1) * 512],
                                start=True, stop=False,
                            )
                            nc.tensor.matmul(
                                pr[:],
                                wproj2_sb[:, oc * 128:(oc + 1) * 128],
                                aT2[:, nb * 512:(nb + 1) * 512],
                                start=False, stop=True,
                            )
                            ob = opool.tile([128, 512], F32, tag="ob")
                            nc.vector.tensor_copy(ob[:], pr[:])
                            nc.sync.dma_start(
                                outT[oc * 128:(oc + 1) * 128,
                                     nb * 512:(nb + 1) * 512],
                                ob[:],
                            )

                    for qb in range(NQB):
                        for h in range(NH):
                            do_tail(h, qb)
                        if debug_dump and qb == NQB - 1:
                            nc.sync.dma_start(dbg["aT01"][:], aT01[:])
                            nc.sync.dma_start(dbg["aT2"][:], aT2[:])
                        do_proj(2 * qb)
                        do_proj(2 * qb + 1)
                ctx_ra.__exit__(None, None, None)
                ctx_up.__exit__(None, None, None)

    nc.compile()
    return nc


def _pack_core_inputs(core, x, W_qkv, b_qkv, W_proj):
    b = core // 4
    heads = [3 * (core % 4) + i for i in range(NH)]
    f32 = np.float32
    bf16 = ml_dtypes.bfloat16

    xT = np.ascontiguousarray(x[b].T).astype(bf16)

    def wcols(kind, h):  # kind 0=q 1=k 2=v
        return W_qkv[:, kind * D + h * HD: kind * D + (h + 1) * HD]

    wp_full = np.concatenate(
        [
            wcols(0, heads[0]), wcols(0, heads[1]),
            wcols(1, heads[0]), wcols(1, heads[1]),
            wcols(0, heads[2]), wcols(1, heads[2]),
            wcols(2, heads[0]), wcols(2, heads[1]),
            wcols(2, heads[2]),
        ],
        axis=1,
    )  # [768, 576]
    wp = np.ascontiguousarray(wp_full.reshape(DC, 128, 576)).astype(bf16)

    def bcols(kind, h):
        return b_qkv[kind * D + h * HD: kind * D + (h + 1) * HD]

    z = np.zeros(HD, f32)
    bp_ = np.stack(
        [
            np.concatenate([bcols(0, heads[0]), bcols(0, heads[1])]),
            np.concatenate([bcols(1, heads[0]), bcols(1, heads[1])]),
            np.concatenate([bcols(0, heads[2]), z]),
            np.concatenate([bcols(1, heads[2]), z]),
            np.concatenate([bcols(2, heads[0]), bcols(2, heads[1])]),
            np.concatenate([bcols(2, heads[2]), z]),
        ],
        axis=1,
    ).astype(f32)  # [128, 6]

    wproj01 = np.concatenate(
        [W_proj[heads[0] * HD:(heads[0] + 1) * HD, :],
         W_proj[heads[1] * HD:(heads[1] + 1) * HD, :]], axis=0
    ).astype(bf16)  # [128, 768]
    wproj2 = np.ascontiguousarray(
        W_proj[heads[2] * HD:(heads[2] + 1) * HD, :]
    ).astype(bf16)  # [64, 768]

    sel = np.zeros((NT, NT, HD), dtype=bf16)
    for i in range(NT):
        sel[i, i, :] = 1.0

    return {
        "xT": xT,
        "wp": wp,
        "bp": np.ascontiguousarray(bp_),
        "wproj01": np.ascontiguousarray(wproj01),
        "wproj2": wproj2,
        "ident": np.eye(128, dtype=bf16),
        "sel": sel,
    }


_MODULE_CACHE = []


def _get_module() -> bass.Bass:
    if not _MODULE_CACHE:
        _MODULE_CACHE.append(build_module())
    return _MODULE_CACHE[0]


def kernel(x, W_qkv, b_qkv, W_proj, b_proj, _trace=False, _result_box=None):
    x = np.asarray(x, np.float32)
    W_qkv = np.asarray(W_qkv, np.float32)
    b_qkv = np.asarray(b_qkv, np.float32)
    W_proj = np.asarray(W_proj, np.float32)
    b_proj = np.asarray(b_proj, np.float32)

    nc = _get_module()
    in_maps = [
        _pack_core_inputs(c, x, W_qkv, b_qkv, W_proj) for c in range(NCORES)
    ]
    res = run_bass_kernel_spmd(nc, in_maps, list(range(NCORES)), trace=_trace)
    if _result_box is not None:
        _result_box.append(res)

    out = np.zeros((B, N, D), np.float32)
    for c in range(NCORES):
        out[c // 4] += res.results[c]["outT"].T
    out += b_proj
    return out
